# revision 11
# baseline (speedup 1.0000x reference)
"""Trainium2 Bass kernel for nn_DecoderRNN (teacher-forced LSTM decoder).

Sharding: pure data-parallel over batch. Each of the 8 NeuronCores gets 8
sequences and runs the full pipeline for them:
  phase A: XW = relu(emb[tok]) @ W_ih.T + (b_ih + b_hh)     (batched matmul)
  phase B: 60-step LSTM recurrence (h @ W_hh.T streamed through the PE with
           4-way column-group tiling, since the per-core batch M=8 is thin;
           the precomputed XW term is injected into PSUM via an identity
           matmul so no separate vector add is needed)
  phase C: logits = hs @ W_out.T + b_out ; log_softmax ; write [8,60,V] f32
           (b_out is partition-broadcast by DMA and fused into the fp8
           logit store; sum(exp) accumulates on the scalar engine)

No collectives. Host side only does input sharding / layout prep (gather of
embedding rows, transposes, dtype casts) and output concatenation.
"""

import sys

import numpy as np

for _p in ("/opt/trn_rl_repo",):
    if _p not in sys.path:
        sys.path.append(_p)

import ml_dtypes  # noqa: E402
from concourse import bacc, mybir, tile  # noqa: E402
from concourse.bass_utils import run_bass_kernel_spmd  # noqa: E402
from concourse.masks import make_identity  # noqa: E402

F32 = mybir.dt.float32
F16 = mybir.dt.float16
BF16 = mybir.dt.bfloat16
FP8 = mybir.dt.float8e4
NPBF16 = ml_dtypes.bfloat16

B, T, H, V = 64, 60, 1024, 32000
NCORES = 8
BL = B // NCORES            # 8 sequences per core
NTOK = T * BL               # 480 tokens per core, tok = t*BL + b
KC = H // 128               # 8 contraction chunks
GATE_PERM = [0, 2, 1, 3]    # (i,f,g,o) -> (i,g,f,o): {i,g} in cols 0:512
MS = [128, 128, 128, 96]    # token-tile sizes (480 = 3*128 + 96)
NV = 63                     # vocab tiles: 62*512 + 256
VS = [512] * 62 + [256]
KORD = [0, 2, 4, 6, 1, 3, 5, 7]  # even chunks first (ready after 1st transpose)

_PROG = None  # cached compiled program


def _build_program():
    nc = bacc.Bacc("TRN2", target_bir_lowering=False, debug=False,
                   num_devices=NCORES)
    d = {}
    d["xT"] = nc.dram_tensor("xT", [H, NTOK], BF16, kind="ExternalInput").ap()
    d["wih"] = nc.dram_tensor("wih", [H, 4 * H], BF16, kind="ExternalInput").ap()
    d["bsum"] = nc.dram_tensor("bsum", [1, 4 * H], BF16, kind="ExternalInput").ap()
    d["whh"] = nc.dram_tensor("whh", [H, 4 * H], BF16, kind="ExternalInput").ap()
    d["hT0"] = nc.dram_tensor("hT0", [128, KC * BL], BF16, kind="ExternalInput").ap()
    d["c0q"] = nc.dram_tensor("c0q", [128, 256], F32, kind="ExternalInput").ap()
    d["woutT"] = nc.dram_tensor("woutT", [H, V], BF16, kind="ExternalInput").ap()
    d["bout"] = nc.dram_tensor("bout", [1, V], BF16, kind="ExternalInput").ap()
    d["out_lp"] = nc.dram_tensor("out_lp", [BL, T, V], F16, kind="ExternalOutput").ap()
    d["h_T"] = nc.dram_tensor("h_T", [BL, H], F32, kind="ExternalOutput").ap()
    d["c_T"] = nc.dram_tensor("c_T", [BL, H], F32, kind="ExternalOutput").ap()

    with tile.TileContext(nc) as tc:
        _body(tc, d)
    nc.compile()
    return nc


def _body(tc, d):
    nc = tc.nc
    Sig = mybir.ActivationFunctionType.Sigmoid
    Tanh = mybir.ActivationFunctionType.Tanh
    Exp = mybir.ActivationFunctionType.Exp
    Ln = mybir.ActivationFunctionType.Ln

    with tc.tile_pool(name="persist", bufs=1) as pp:
        hsT = pp.tile([128, KC * NTOK], BF16, tag="hsT")
        sume = pp.tile([128, 4 * NV], F32, tag="sume")
        ident = pp.tile([128, 128], F32, tag="ident")
        id8b = pp.tile([8, 8], BF16, tag="id8b")
        id8s = pp.tile([128, 8], BF16, tag="id8s")
        ones_x = pp.tile([1, 128], BF16, tag="ones_x")
        hT0_sb = pp.tile([128, KC * BL], BF16, tag="hT0")
        make_identity(nc, ident[:])
        nc.vector.tensor_copy(id8b[:], ident[0:8, 0:8])
        for q in range(4):
            # identity block at each col-group's partition base (matmul
            # requires lhsT and rhs to start at the same partition)
            nc.sync.dma_start(id8s[32 * q:32 * q + 8, :], id8b[:])
        nc.vector.memset(ones_x[:], 1.0)
        nc.sync.dma_start(hT0_sb[:], d["hT0"][:])

        with tc.tile_pool(name="ab", bufs=1) as pab:
            XW = [pab.tile([128, 4 * H], BF16, tag=f"xw{m}", name=f"xw{m}")
                  for m in range(4)]

            # ---------------- phase A: XW = xT.T @ W_ihT + b ----------------
            with (
                tc.tile_pool(name="pa", bufs=1) as pa,
                tc.tile_pool(name="psA", bufs=4, space="PSUM") as psA,
            ):
                xT_sb = [pa.tile([128, NTOK], BF16, tag=f"xt{k}",
                                 name=f"xt{k}") for k in range(KC)]
                wih_sb = [pa.tile([128, 4 * H], BF16, tag=f"wih{k}",
                                  name=f"wih{k}") for k in range(KC)]
                bsum_sb = pa.tile([1, 4 * H], BF16, tag="bsum")
                nc.sync.dma_start(bsum_sb[:], d["bsum"][:])
                for k in range(KC):
                    nc.sync.dma_start(xT_sb[k][:],
                                      d["xT"][k * 128:(k + 1) * 128, :])
                    nc.sync.dma_start(wih_sb[k][:],
                                      d["wih"][k * 128:(k + 1) * 128, :])
                for m in range(4):
                    ms = MS[m]
                    for n in range(8):
                        ps = psA.tile([128, 512], F32, tag="psA")
                        for k in range(KC):
                            nc.tensor.matmul(
                                ps[:ms], xT_sb[k][:, m * 128:m * 128 + ms],
                                wih_sb[k][:, n * 512:(n + 1) * 512],
                                start=(k == 0), stop=False)
                        nc.tensor.matmul(
                            ps[:ms], ones_x[:, :ms],
                            bsum_sb[:, n * 512:(n + 1) * 512],
                            start=False, stop=True)
                        nc.vector.tensor_copy(
                            XW[m][:ms, n * 512:(n + 1) * 512], ps[:ms])

            # ---------------- phase B: LSTM recurrence ----------------------
            with (
                tc.tile_pool(name="pb", bufs=1) as pb,
                tc.tile_pool(name="pbs", bufs=2) as pbs,
                tc.tile_pool(name="pbw", bufs=3) as pbw,
                tc.tile_pool(name="psB", bufs=1, space="PSUM") as psB,
                tc.tile_pool(name="psT", bufs=2, space="PSUM") as psT,
            ):
                whh_sb = [pb.tile([128, 4 * H], BF16, tag=f"whh{k}",
                                  name=f"whh{k}") for k in range(KC)]
                for k in range(KC):
                    nc.sync.dma_start(whh_sb[k][:],
                                      d["whh"][k * 128:(k + 1) * 128, :])
                c_prev = pbs.tile([128, 256], F32, tag="c")
                nc.sync.dma_start(c_prev[:], d["c0q"][:])
                # persistent psum accumulators (2 gate-halves x ping-pong),
                # memset once so junk partitions always hold finite zeros
                gat = [[psB.tile([128, 512], F32, tag=f"g{h}{i}",
                                 name=f"g{h}{i}") for i in range(2)]
                       for h in range(2)]
                for h in range(2):
                    for i in range(2):
                        nc.vector.memset(gat[h][i][:], 0.0)
                warm_ps = psB.tile([8, 128], F32, tag="warm")

                h_q = None
                for t in range(T):
                    m, p0 = t // 16, (t % 16) * 8
                    # xw_t: [128,1024] quarter layout from XW[m]
                    xw = pbw.tile([128, 1024], BF16, tag="xwt")
                    for q in range(4):
                        src = XW[m][p0:p0 + 8, :].rearrange(
                            "p (g q j) -> p g q j", g=4, q=4)[:, :, q, :]
                        dst = xw[32 * q:32 * q + 8, :].rearrange(
                            "p (g j) -> p g j", g=4)
                        nc.sync.dma_start(dst, src)

                    g0 = gat[0][t % 2]
                    g1 = gat[1][t % 2]
                    for nh, gps in ((0, g0), (1, g1)):
                        for q in range(4):  # inject XW via identity matmul
                            nc.tensor.matmul(
                                gps[32 * q:32 * q + 8, :],
                                id8s[32 * q:32 * q + 8, :],
                                xw[32 * q:32 * q + 8,
                                   nh * 512:(nh + 1) * 512],
                                start=True, stop=False,
                                skip_group_check=True,
                                tile_position=(32 * q, 32 * q))
                        for ki, k in enumerate(KORD):
                            if t == 0:
                                lhsT = hT0_sb[:, k * BL:(k + 1) * BL]
                            else:
                                off = k * NTOK + (t - 1) * BL
                                lhsT = hsT[:, off:off + BL]
                            for q in range(4):
                                rhs = whh_sb[k][:, q * 1024 + nh * 512:
                                                q * 1024 + (nh + 1) * 512]
                                nc.tensor.matmul(
                                    gps[32 * q:32 * q + 8, :],
                                    lhsT, rhs,
                                    start=False, stop=(ki == KC - 1),
                                    skip_group_check=True,
                                    tile_position=(0, 32 * q))
                    # cols of g0: 0:256 = i, 256:512 = g ; g1: f, o
                    i_t = pbs.tile([128, 256], F32, tag="i_t")
                    g_t = pbs.tile([128, 256], F32, tag="g_t")
                    f_t = pbs.tile([128, 256], F32, tag="f_t")
                    o_t = pbs.tile([128, 256], F32, tag="o_t")
                    t1 = pbs.tile([128, 256], F32, tag="t1")
                    t2 = pbs.tile([128, 256], F32, tag="t2")
                    th = pbs.tile([128, 256], F32, tag="th")
                    c_new = pbs.tile([128, 256], F32, tag="c")
                    h_q = pbs.tile([128, 256], F32, tag="hq")
                    nc.scalar.activation(i_t[:], g0[:, 0:256], Sig)
                    nc.scalar.activation(g_t[:], g0[:, 256:512], Tanh)
                    nc.vector.tensor_mul(t1[:], i_t[:], g_t[:])
                    nc.scalar.activation(f_t[:], g1[:, 0:256], Sig)
                    nc.vector.tensor_mul(t2[:], f_t[:], c_prev[:])
                    nc.scalar.activation(o_t[:], g1[:, 256:512], Sig)
                    nc.vector.tensor_add(c_new[:], t1[:], t2[:])
                    # keep the PE activity monitor warm through the gap
                    nc.tensor.matmul(warm_ps[:], ident[0:8, 0:8],
                                     c_new[0:8, 0:128], start=True, stop=True,
                                     skip_group_check=True)
                    nc.scalar.activation(th[:], c_new[:], Tanh)
                    nc.vector.tensor_mul(h_q[:], o_t[:], th[:])
                    nc.tensor.matmul(warm_ps[:], ident[0:8, 0:8],
                                     th[0:8, 0:128], start=True, stop=True,
                                     skip_group_check=True)
                    # transpose h -> hsT slots (even chunks first)
                    tp = psT.tile([128, 256], F32, tag="tp")
                    for half in range(2):
                        nc.tensor.transpose(
                            tp[:, half * 128:(half + 1) * 128],
                            h_q[:, half * 128:(half + 1) * 128], ident[:])
                        src = tp[:, half * 128:(half + 1) * 128].rearrange(
                            "p (q c) -> p q c", q=4)[:, :, 0:8]
                        dst = hsT[:].rearrange(
                            "p (k s) -> p k s", k=KC)[:, half::2,
                                                      t * BL:(t + 1) * BL]
                        nc.vector.tensor_copy(dst, src)
                    c_prev = c_new

                # final h/c outputs (layout [32q+b, j'] -> [b, q*256+j'])
                for q in range(4):
                    nc.sync.dma_start(d["h_T"][:, q * 256:(q + 1) * 256],
                                      h_q[32 * q:32 * q + 8, :])
                    nc.sync.dma_start(d["c_T"][:, q * 256:(q + 1) * 256],
                                      c_prev[32 * q:32 * q + 8, :])

        # ---------------- phase C: projection + log_softmax -----------------
        with (
            tc.tile_pool(name="pc", bufs=1) as pc,
            tc.tile_pool(name="pcw", bufs=18) as pcw,
            tc.tile_pool(name="pcs", bufs=3) as pcs,
            tc.tile_pool(name="pst", bufs=6) as pst,
            tc.tile_pool(name="psC", bufs=4, space="PSUM") as psC,
        ):
            lg8 = pc.tile([128, 4 * V], FP8, tag="lg8")
            for v in range(NV):
                vs = VS[v]
                wv = [pcw.tile([128, 512], BF16, tag="wout", name="woutT")
                      for _ in range(KC)]
                for k in range(KC):
                    nc.sync.dma_start(
                        wv[k][:, :vs],
                        d["woutT"][k * 128:(k + 1) * 128, v * 512:v * 512 + vs])
                bb = pcw.tile([128, 512], BF16, tag="bb")
                nc.sync.dma_start(
                    bb[:, :vs],
                    d["bout"][:, v * 512:v * 512 + vs].to_broadcast([128, vs]))
                for m in range(4):
                    ms = MS[m]
                    ps = psC.tile([128, 512], F32, tag="psC")
                    for k in range(KC):
                        off = k * NTOK + m * 128
                        nc.tensor.matmul(ps[:ms, :vs], hsT[:, off:off + ms],
                                         wv[k][:, :vs],
                                         start=(k == 0), stop=(k == KC - 1))
                    lgs = lg8[:ms, m * V + v * 512:m * V + v * 512 + vs]
                    nc.vector.tensor_add(lgs, ps[:ms, :vs], bb[:ms, :vs])
                    ex = pcs.tile([128, 512], BF16, tag="ex")
                    nc.scalar.activation(
                        ex[:ms, :vs], lgs, Exp,
                        accum_out=sume[:ms, m * NV + v:m * NV + v + 1])
            # pass 2: lse + subtract + write out
            for m in range(4):
                ms = MS[m]
                ntp = ms // 8
                s = pc.tile([128, 1], F32, tag=f"s{m}")
                nc.vector.tensor_reduce(
                    s[:ms], sume[:ms, m * NV:(m + 1) * NV],
                    axis=mybir.AxisListType.X, op=mybir.AluOpType.add)
                lse = pc.tile([128, 1], F32, tag=f"lse{m}")
                nc.scalar.activation(lse[:ms], s[:ms], Ln)
                for v in range(NV):
                    vs = VS[v]
                    st = pst.tile([128, 512], F16, tag="st")
                    eng = nc.vector if v % 2 == 0 else nc.gpsimd
                    eng.tensor_scalar(
                        st[:ms, :vs],
                        lg8[:ms, m * V + v * 512:m * V + v * 512 + vs],
                        lse[:ms], None, op0=mybir.AluOpType.subtract)
                    dst = d["out_lp"][:, m * 16:m * 16 + ntp,
                                      v * 512:v * 512 + vs].rearrange(
                                          "b t v -> t b v")
                    nc.sync.dma_start(dst, st[:ms, :vs])


# --------------------------- host-side prep ---------------------------------

def _core_inputs(k, tokens, emb, wihT_s, bsum_s, whh_s, h0, c0, woutT, bout):
    tok_k = tokens[k * BL:(k + 1) * BL]                     # [8, 60]
    x = emb[tok_k]                                          # [8, 60, H] f32
    np.maximum(x, 0.0, out=x)
    x_t = np.ascontiguousarray(x.transpose(1, 0, 2)).reshape(NTOK, H)
    xT = np.ascontiguousarray(x_t.T)                        # [H, 480]

    h0k = h0[0, k * BL:(k + 1) * BL]                        # [8, H]
    hT0 = np.ascontiguousarray(
        h0k.reshape(BL, KC, 128).transpose(2, 1, 0)).reshape(128, KC * BL)

    c0k = c0[0, k * BL:(k + 1) * BL]                        # [8, H]
    c0q = np.zeros((4, 32, 256), np.float32)
    c0q[:, :BL] = c0k.reshape(BL, 4, 256).transpose(1, 0, 2)
    c0q = c0q.reshape(128, 256)

    return {
        "xT": xT.astype(NPBF16),
        "wih": wihT_s,
        "bsum": bsum_s,
        "whh": whh_s,
        "hT0": hT0.astype(NPBF16),
        "c0q": c0q,
        "woutT": woutT,
        "bout": bout,
    }


def prep_all_inputs(encoder_outputs, h0, c0, target_tensor, emb, W_ih, W_hh,
                    b_ih, b_hh, W_out, b_out):
    h0 = np.asarray(h0, np.float32)
    c0 = np.asarray(c0, np.float32)
    tgt = np.asarray(target_tensor)
    emb = np.asarray(emb, np.float32)
    W_ih = np.asarray(W_ih, np.float32)
    W_hh = np.asarray(W_hh, np.float32)
    b_ih = np.asarray(b_ih, np.float32)
    b_hh = np.asarray(b_hh, np.float32)
    W_out = np.asarray(W_out, np.float32)
    b_out = np.asarray(b_out, np.float32)

    tokens = np.concatenate(
        [np.zeros((B, 1), np.int64), tgt[:, :T - 1]], axis=1).astype(np.int64)

    # W_ih.T with output cols permuted to gate order (i,g,f,o)
    wihT_s = np.ascontiguousarray(
        W_ih.reshape(4, H, H)[GATE_PERM].reshape(4 * H, H).T).astype(NPBF16)
    bsum_s = ((b_ih + b_hh).reshape(4, H)[GATE_PERM].reshape(1, 4 * H)
              .astype(NPBF16))
    # W_hh.T in stream layout: col = q*1024 + gate'*256 + j'
    whh_s = np.ascontiguousarray(
        W_hh.T.reshape(H, 4, 4, 256)[:, GATE_PERM].transpose(0, 2, 1, 3)
        .reshape(H, 4 * H)).astype(NPBF16)
    woutT = np.ascontiguousarray(W_out.T).astype(NPBF16)
    boutr = b_out.reshape(1, V).astype(NPBF16)

    return [
        _core_inputs(k, tokens, emb, wihT_s, bsum_s, whh_s, h0, c0, woutT,
                     boutr)
        for k in range(NCORES)
    ]


def assemble_outputs(results):
    log_probs = np.concatenate([r["out_lp"] for r in results], axis=0).astype(np.float32)
    h_T = np.concatenate([r["h_T"] for r in results], axis=0)[None]
    c_T = np.concatenate([r["c_T"] for r in results], axis=0)[None]
    return log_probs, h_T, c_T


def get_program():
    global _PROG
    if _PROG is None:
        _PROG = _build_program()
    return _PROG


def kernel(**inputs):
    nc = get_program()
    in_maps = prep_all_inputs(**inputs)
    res = run_bass_kernel_spmd(nc, in_maps, list(range(NCORES)))
    return assemble_outputs(res.results)


# revision 12
# speedup vs baseline: 1.6819x; 1.6819x over previous
"""Trainium2 Bass kernel for nn_DecoderRNN (teacher-forced LSTM decoder).

Sharding: pure data-parallel over batch. Each of the 8 NeuronCores gets 8
sequences and runs the full pipeline for them:
  phase A: XW = relu(emb[tok]) @ W_ih.T + (b_ih + b_hh)     (batched matmul)
  phase B: 60-step LSTM recurrence (h @ W_hh.T streamed through the PE with
           4-way column-group tiling, since the per-core batch M=8 is thin;
           the precomputed XW term is injected into PSUM via an identity
           matmul so no separate vector add is needed)
  phase C: logits = hs @ W_out.T + b_out ; log_softmax ; write [8,60,V] f32
           (b_out is partition-broadcast by DMA and fused into the fp8
           logit store; sum(exp) accumulates on the scalar engine)

No collectives. Host side only does input sharding / layout prep (gather of
embedding rows, transposes, dtype casts) and output concatenation.
"""

import sys

import numpy as np

for _p in ("/opt/trn_rl_repo",):
    if _p not in sys.path:
        sys.path.append(_p)

import ml_dtypes  # noqa: E402
from concourse import bacc, mybir, tile  # noqa: E402
from concourse.bass_utils import run_bass_kernel_spmd  # noqa: E402
from concourse.masks import make_identity  # noqa: E402

F32 = mybir.dt.float32
F16 = mybir.dt.float16
BF16 = mybir.dt.bfloat16
FP8 = mybir.dt.float8e4
NPBF16 = ml_dtypes.bfloat16

B, T, H, V = 64, 60, 1024, 32000
NCORES = 8
BL = B // NCORES            # 8 sequences per core
NTOK = T * BL               # 480 tokens per core, tok = t*BL + b
KC = H // 128               # 8 contraction chunks
GATE_PERM = [0, 2, 1, 3]    # (i,f,g,o) -> (i,g,f,o): {i,g} in cols 0:512
MS = [128, 128, 128, 96]    # token-tile sizes (480 = 3*128 + 96)
NV = 63                     # vocab tiles: 62*512 + 256
VS = [512] * 62 + [256]
KORD = [0, 2, 4, 6, 1, 3, 5, 7]  # even chunks first (ready after 1st transpose)

_PROG = None  # cached compiled program


def _build_program():
    nc = bacc.Bacc("TRN2", target_bir_lowering=False, debug=False,
                   num_devices=NCORES)
    d = {}
    d["xT"] = nc.dram_tensor("xT", [H, NTOK], BF16, kind="ExternalInput").ap()
    d["wih"] = nc.dram_tensor("wih", [H, 4 * H], BF16, kind="ExternalInput").ap()
    d["bsum"] = nc.dram_tensor("bsum", [1, 4 * H], BF16, kind="ExternalInput").ap()
    d["whh"] = nc.dram_tensor("whh", [H, 4 * H], BF16, kind="ExternalInput").ap()
    d["hT0"] = nc.dram_tensor("hT0", [128, KC * BL], BF16, kind="ExternalInput").ap()
    d["c0q"] = nc.dram_tensor("c0q", [128, 256], F32, kind="ExternalInput").ap()
    d["woutT"] = nc.dram_tensor("woutT", [H, V], BF16, kind="ExternalInput").ap()
    d["bout"] = nc.dram_tensor("bout", [1, V], BF16, kind="ExternalInput").ap()
    d["out_lp"] = nc.dram_tensor("out_lp", [BL, T, V], F32, kind="ExternalOutput").ap()
    d["h_T"] = nc.dram_tensor("h_T", [BL, H], F32, kind="ExternalOutput").ap()
    d["c_T"] = nc.dram_tensor("c_T", [BL, H], F32, kind="ExternalOutput").ap()

    with tile.TileContext(nc) as tc:
        _body(tc, d)
    nc.compile()
    return nc


def _body(tc, d):
    nc = tc.nc
    Sig = mybir.ActivationFunctionType.Sigmoid
    Tanh = mybir.ActivationFunctionType.Tanh
    Exp = mybir.ActivationFunctionType.Exp
    Ln = mybir.ActivationFunctionType.Ln

    with tc.tile_pool(name="persist", bufs=1) as pp:
        hsT = pp.tile([128, KC * 512], BF16, tag="hsT")
        sume = pp.tile([128, 4 * NV], F32, tag="sume")
        ident = pp.tile([128, 128], F32, tag="ident")
        id32b = pp.tile([32, 32], BF16, tag="id32b")
        id32s = pp.tile([128, 32], BF16, tag="id32s")
        ones_x = pp.tile([1, 128], BF16, tag="ones_x")
        make_identity(nc, ident[:])
        nc.vector.tensor_copy(id32b[:], ident[0:32, 0:32])
        for q in range(4):
            # identity block at each col-group's partition base (matmul
            # requires lhsT and rhs to start at the same partition)
            nc.sync.dma_start(id32s[32 * q:32 * q + 32, :], id32b[:])
        nc.vector.memset(ones_x[:], 1.0)
        # hsT slot layout per chunk k: col k*512 + (t+4)*8 + b; slots
        # t=-4..-1 are the zeroed prefix, with h0 in slot -1. Matmul lhsT
        # reads 4-slot windows [t-4..t-1] as a [128,32] stationary operand
        # so the whole PE array looks busy (keeps the HAM clock warm).
        nc.vector.memset(hsT[:], 0.0)
        nc.sync.dma_start(
            hsT[:].rearrange("p (k s) -> p k s", k=KC)[:, :, 24:32],
            d["hT0"][:].rearrange("p (k b) -> p k b", k=KC))

        with tc.tile_pool(name="ab", bufs=1) as pab:
            XW = [pab.tile([128, 4 * H], BF16, tag=f"xw{m}", name=f"xw{m}")
                  for m in range(4)]

            # ---------------- phase A: XW = xT.T @ W_ihT + b ----------------
            with (
                tc.tile_pool(name="pa", bufs=1) as pa,
                tc.tile_pool(name="psA", bufs=4, space="PSUM") as psA,
            ):
                xT_sb = [pa.tile([128, NTOK], BF16, tag=f"xt{k}",
                                 name=f"xt{k}") for k in range(KC)]
                wih_sb = [pa.tile([128, 4 * H], BF16, tag=f"wih{k}",
                                  name=f"wih{k}") for k in range(KC)]
                bsum_sb = pa.tile([1, 4 * H], BF16, tag="bsum")
                nc.sync.dma_start(bsum_sb[:], d["bsum"][:])
                for k in range(KC):
                    nc.sync.dma_start(xT_sb[k][:],
                                      d["xT"][k * 128:(k + 1) * 128, :])
                    nc.sync.dma_start(wih_sb[k][:],
                                      d["wih"][k * 128:(k + 1) * 128, :])
                for m in range(4):
                    ms = MS[m]
                    for n in range(8):
                        ps = psA.tile([128, 512], F32, tag="psA")
                        for k in range(KC):
                            nc.tensor.matmul(
                                ps[:ms], xT_sb[k][:, m * 128:m * 128 + ms],
                                wih_sb[k][:, n * 512:(n + 1) * 512],
                                start=(k == 0), stop=False)
                        nc.tensor.matmul(
                            ps[:ms], ones_x[:, :ms],
                            bsum_sb[:, n * 512:(n + 1) * 512],
                            start=False, stop=True)
                        nc.vector.tensor_copy(
                            XW[m][:ms, n * 512:(n + 1) * 512], ps[:ms])

            # ---------------- phase B: LSTM recurrence ----------------------
            with (
                tc.tile_pool(name="pb", bufs=1) as pb,
                tc.tile_pool(name="pbs", bufs=2) as pbs,
                tc.tile_pool(name="pbw", bufs=3) as pbw,
                tc.tile_pool(name="psB", bufs=1, space="PSUM") as psB,
                tc.tile_pool(name="psT", bufs=2, space="PSUM") as psT,
            ):
                whh_sb = [pb.tile([128, 4 * H], BF16, tag=f"whh{k}",
                                  name=f"whh{k}") for k in range(KC)]
                for k in range(KC):
                    nc.sync.dma_start(whh_sb[k][:],
                                      d["whh"][k * 128:(k + 1) * 128, :])
                c_prev = pbs.tile([128, 256], F32, tag="c")
                nc.sync.dma_start(c_prev[:], d["c0q"][:])
                # persistent psum accumulators (2 gate-halves x ping-pong),
                # memset once so junk partitions always hold finite zeros
                gat = [[psB.tile([128, 512], F32, tag=f"g{h}{i}",
                                 name=f"g{h}{i}") for i in range(2)]
                       for h in range(2)]
                xw_bufs = [pbw.tile([128, 1024], BF16, tag=f"xwb{i}",
                                    name=f"xwb{i}") for i in range(3)]
                for i in range(3):
                    nc.vector.memset(xw_bufs[i][:], 0.0)

                h_q = None
                for t in range(T):
                    m, p0 = t // 16, (t % 16) * 8
                    # xw_t: [128,1024] quarter layout from XW[m], valid rows
                    # at partition 32q+24+b
                    xw = xw_bufs[t % 3]
                    for q in range(4):
                        src = XW[m][p0:p0 + 8, :].rearrange(
                            "p (g q j) -> p g q j", g=4, q=4)[:, :, q, :]
                        dst = xw[32 * q + 24:32 * q + 32, :].rearrange(
                            "p (g j) -> p g j", g=4)
                        nc.sync.dma_start(dst, src)

                    g0 = gat[0][t % 2]
                    g1 = gat[1][t % 2]
                    for nh, gps in ((0, g0), (1, g1)):
                        for q in range(4):  # inject XW via identity matmul
                            nc.tensor.matmul(
                                gps[32 * q:32 * q + 32, :],
                                id32s[32 * q:32 * q + 32, :],
                                xw[32 * q:32 * q + 32,
                                   nh * 512:(nh + 1) * 512],
                                start=True, stop=False,
                                skip_group_check=True,
                                tile_position=(32 * q, 32 * q))
                        for ki, k in enumerate(KORD):
                            lhsT = hsT[:, k * 512 + t * 8:k * 512 + t * 8 + 32]
                            for q in range(4):
                                rhs = whh_sb[k][:, q * 1024 + nh * 512:
                                                q * 1024 + (nh + 1) * 512]
                                nc.tensor.matmul(
                                    gps[32 * q:32 * q + 32, :],
                                    lhsT, rhs,
                                    start=False, stop=(ki == KC - 1),
                                    skip_group_check=True,
                                    tile_position=(0, 32 * q))
                    # cols of g0: 0:256 = i, 256:512 = g ; g1: f, o
                    i_t = pbs.tile([128, 256], F32, tag="i_t")
                    g_t = pbs.tile([128, 256], F32, tag="g_t")
                    f_t = pbs.tile([128, 256], F32, tag="f_t")
                    o_t = pbs.tile([128, 256], F32, tag="o_t")
                    t1 = pbs.tile([128, 256], F32, tag="t1")
                    t2 = pbs.tile([128, 256], F32, tag="t2")
                    th = pbs.tile([128, 256], F32, tag="th")
                    c_new = pbs.tile([128, 256], F32, tag="c")
                    h_q = pbs.tile([128, 256], F32, tag="hq")
                    nc.scalar.activation(i_t[:], g0[:, 0:256], Sig)
                    nc.scalar.activation(g_t[:], g0[:, 256:512], Tanh)
                    nc.vector.tensor_mul(t1[:], i_t[:], g_t[:])
                    nc.scalar.activation(f_t[:], g1[:, 0:256], Sig)
                    nc.vector.tensor_mul(t2[:], f_t[:], c_prev[:])
                    nc.scalar.activation(o_t[:], g1[:, 256:512], Sig)
                    nc.vector.tensor_add(c_new[:], t1[:], t2[:])
                    nc.scalar.activation(th[:], c_new[:], Tanh)
                    nc.vector.tensor_mul(h_q[:], o_t[:], th[:])
                    # transpose h -> hsT slot t (even chunks first); valid h
                    # lives at partitions 32q+24..32q+32
                    tp = psT.tile([128, 256], F32, tag="tp")
                    for half in range(2):
                        nc.tensor.transpose(
                            tp[:, half * 128:(half + 1) * 128],
                            h_q[:, half * 128:(half + 1) * 128], ident[:])
                        src = tp[:, half * 128:(half + 1) * 128].rearrange(
                            "p (q c) -> p q c", q=4)[:, :, 24:32]
                        dst = hsT[:].rearrange(
                            "p (k s) -> p k s",
                            k=KC)[:, half::2, (t + 4) * 8:(t + 5) * 8]
                        nc.vector.tensor_copy(dst, src)
                    c_prev = c_new

                # final h/c outputs (layout [32q+b, j'] -> [b, q*256+j'])
                for q in range(4):
                    nc.sync.dma_start(d["h_T"][:, q * 256:(q + 1) * 256],
                                      h_q[32 * q + 24:32 * q + 32, :])
                    nc.sync.dma_start(d["c_T"][:, q * 256:(q + 1) * 256],
                                      c_prev[32 * q + 24:32 * q + 32, :])

        # ---------------- phase C: projection + log_softmax -----------------
        with (
            tc.tile_pool(name="pc", bufs=1) as pc,
            tc.tile_pool(name="pcw", bufs=18) as pcw,
            tc.tile_pool(name="pcs", bufs=3) as pcs,
            tc.tile_pool(name="pst", bufs=6) as pst,
            tc.tile_pool(name="psC", bufs=4, space="PSUM") as psC,
        ):
            lg8 = pc.tile([128, 4 * V], FP8, tag="lg8")
            for v in range(NV):
                vs = VS[v]
                wv = [pcw.tile([128, 512], BF16, tag="wout", name="woutT")
                      for _ in range(KC)]
                for k in range(KC):
                    nc.sync.dma_start(
                        wv[k][:, :vs],
                        d["woutT"][k * 128:(k + 1) * 128, v * 512:v * 512 + vs])
                bb = pcw.tile([128, 512], BF16, tag="bb")
                nc.sync.dma_start(
                    bb[:, :vs],
                    d["bout"][:, v * 512:v * 512 + vs].to_broadcast([128, vs]))
                for m in range(4):
                    ms = MS[m]
                    ps = psC.tile([128, 512], F32, tag="psC")
                    for k in range(KC):
                        off = k * 512 + 32 + m * 128
                        nc.tensor.matmul(ps[:ms, :vs], hsT[:, off:off + ms],
                                         wv[k][:, :vs],
                                         start=(k == 0), stop=(k == KC - 1))
                    lgs = lg8[:ms, m * V + v * 512:m * V + v * 512 + vs]
                    nc.vector.tensor_add(lgs, ps[:ms, :vs], bb[:ms, :vs])
                    ex = pcs.tile([128, 512], BF16, tag="ex")
                    nc.scalar.activation(
                        ex[:ms, :vs], lgs, Exp,
                        accum_out=sume[:ms, m * NV + v:m * NV + v + 1])
            # pass 2: lse + subtract + write out
            for m in range(4):
                ms = MS[m]
                ntp = ms // 8
                s = pc.tile([128, 1], F32, tag=f"s{m}")
                nc.vector.tensor_reduce(
                    s[:ms], sume[:ms, m * NV:(m + 1) * NV],
                    axis=mybir.AxisListType.X, op=mybir.AluOpType.add)
                lse = pc.tile([128, 1], F32, tag=f"lse{m}")
                nc.scalar.activation(lse[:ms], s[:ms], Ln)
                for v in range(NV):
                    vs = VS[v]
                    st = pst.tile([128, 512], F32, tag="st")
                    nc.vector.tensor_scalar(
                        st[:ms, :vs],
                        lg8[:ms, m * V + v * 512:m * V + v * 512 + vs],
                        lse[:ms], None, op0=mybir.AluOpType.subtract)
                    dst = d["out_lp"][:, m * 16:m * 16 + ntp,
                                      v * 512:v * 512 + vs].rearrange(
                                          "b t v -> t b v")
                    nc.sync.dma_start(dst, st[:ms, :vs])


# --------------------------- host-side prep ---------------------------------

def _core_inputs(k, tokens, emb, wihT_s, bsum_s, whh_s, h0, c0, woutT, bout):
    tok_k = tokens[k * BL:(k + 1) * BL]                     # [8, 60]
    x = emb[tok_k]                                          # [8, 60, H] f32
    np.maximum(x, 0.0, out=x)
    x_t = np.ascontiguousarray(x.transpose(1, 0, 2)).reshape(NTOK, H)
    xT = np.ascontiguousarray(x_t.T)                        # [H, 480]

    h0k = h0[0, k * BL:(k + 1) * BL]                        # [8, H]
    hT0 = np.ascontiguousarray(
        h0k.reshape(BL, KC, 128).transpose(2, 1, 0)).reshape(128, KC * BL)

    c0k = c0[0, k * BL:(k + 1) * BL]                        # [8, H]
    c0q = np.zeros((4, 32, 256), np.float32)
    c0q[:, 24:] = c0k.reshape(BL, 4, 256).transpose(1, 0, 2)
    c0q = c0q.reshape(128, 256)

    return {
        "xT": xT.astype(NPBF16),
        "wih": wihT_s,
        "bsum": bsum_s,
        "whh": whh_s,
        "hT0": hT0.astype(NPBF16),
        "c0q": c0q,
        "woutT": woutT,
        "bout": bout,
    }


def prep_all_inputs(encoder_outputs, h0, c0, target_tensor, emb, W_ih, W_hh,
                    b_ih, b_hh, W_out, b_out):
    h0 = np.asarray(h0, np.float32)
    c0 = np.asarray(c0, np.float32)
    tgt = np.asarray(target_tensor)
    emb = np.asarray(emb, np.float32)
    W_ih = np.asarray(W_ih, np.float32)
    W_hh = np.asarray(W_hh, np.float32)
    b_ih = np.asarray(b_ih, np.float32)
    b_hh = np.asarray(b_hh, np.float32)
    W_out = np.asarray(W_out, np.float32)
    b_out = np.asarray(b_out, np.float32)

    tokens = np.concatenate(
        [np.zeros((B, 1), np.int64), tgt[:, :T - 1]], axis=1).astype(np.int64)

    # W_ih.T with output cols permuted to gate order (i,g,f,o)
    wihT_s = np.ascontiguousarray(
        W_ih.reshape(4, H, H)[GATE_PERM].reshape(4 * H, H).T).astype(NPBF16)
    bsum_s = ((b_ih + b_hh).reshape(4, H)[GATE_PERM].reshape(1, 4 * H)
              .astype(NPBF16))
    # W_hh.T in stream layout: col = q*1024 + gate'*256 + j'
    whh_s = np.ascontiguousarray(
        W_hh.T.reshape(H, 4, 4, 256)[:, GATE_PERM].transpose(0, 2, 1, 3)
        .reshape(H, 4 * H)).astype(NPBF16)
    woutT = np.ascontiguousarray(W_out.T).astype(NPBF16)
    boutr = b_out.reshape(1, V).astype(NPBF16)

    return [
        _core_inputs(k, tokens, emb, wihT_s, bsum_s, whh_s, h0, c0, woutT,
                     boutr)
        for k in range(NCORES)
    ]


def assemble_outputs(results):
    log_probs = np.concatenate([r["out_lp"] for r in results], axis=0)
    h_T = np.concatenate([r["h_T"] for r in results], axis=0)[None]
    c_T = np.concatenate([r["c_T"] for r in results], axis=0)[None]
    return log_probs, h_T, c_T


def get_program():
    global _PROG
    if _PROG is None:
        _PROG = _build_program()
    return _PROG


def kernel(**inputs):
    nc = get_program()
    in_maps = prep_all_inputs(**inputs)
    res = run_bass_kernel_spmd(nc, in_maps, list(range(NCORES)))
    return assemble_outputs(res.results)
